# revision 51
# baseline (speedup 1.0000x reference)
"""MoE top-2 routing kernel for Trainium2, expert-parallel across 8 NeuronCores.

Problem (hardcoded): x [4, 2048, 1024] f32, gate_w [1024, 8], w1 [8, 1024, 4096],
w2 [8, 4096, 1024], TOP_K=2, exact GELU, softmax-renormalized top-2 combine.

Strategy: each core owns one expert. x is replicated; every core computes the
router for all 8192 tokens (f32, exact), compacts the token ids routed to its
own expert with gpsimd sparse_gather, gathers those rows of x by indirect DMA,
runs the expert MLP in bf16 (w1 and w2 both resident in SBUF, loaded once):
hT = gelu(w1.T @ x.T) stays f-major; the down-proj uses hT 128-token blocks as
the stationary operand so psY comes out token-major and feeds the indirect
scatter directly (no output transposes). Output is split into two half-width
tables [NT+1, 512] so the scatter's row stride is 2KB (max DGE byte offset
16.8MB < 2^25: no offset wrap on hardware) and the scatter-out AP is passed as
a 128-row prefix of the table, matching the true 128-row transfer. Host sums
the 8 per-core outputs (each token appears on exactly the two cores of its
top-2 experts).
"""

import numpy as np
from contextlib import ExitStack

import concourse.bass as bass
import concourse.mybir as mybir
import concourse.tile as tile
from concourse import bacc, library_config
from concourse.bass_utils import run_bass_kernel_spmd

P = 128
B, T, C, E, F = 4, 2048, 1024, 8, 4096
NT = B * T              # 8192 tokens
NTILE = NT // P         # 64 token tiles
CB = C // P             # 8 contraction blocks over C
FB = F // P             # 32 blocks over F
CH = C // 2             # half-width output tables
DUMP = NT               # dump row index (x_pad[NT] == 0)

f32 = mybir.dt.float32
i32 = mybir.dt.int32
u32 = mybir.dt.uint32
bf16 = mybir.dt.bfloat16


def _src_salt():
    """Shape-salt derived from this file's source so every kernel edit changes
    the executable signature through every cache layer (client NEFF cache and
    any terminal-side executable cache)."""
    import hashlib

    with open(__file__, "rb") as f:
        h = int(hashlib.sha256(f.read()).hexdigest(), 16)
    return 1 + (h % 509)


def build_nc(cap, repeats=1):
    """cap: per-expert token capacity, multiple of 128. repeats: trace the whole
    program body N times back-to-back; used to measure per-invocation HW time
    by differencing."""
    assert cap % P == 0
    capt = cap // P         # gather tiles
    capw = cap // 16        # wrapped compaction cols
    # pass widths in tiles of 128 tokens, up to 4 tiles (512) per pass. A
    # small (2-tile) first pass starts the MLP sooner and gives the chunked
    # w2 load time to finish before the first down-proj needs it.
    if capt > 6:
        rest = capt - 2
        widths = [2] + [4] * (rest // 4)
        if rest % 4:
            widths.append(rest % 4)
    else:
        widths = [min(4, capt)] + ([capt - 4] if capt > 4 else [])

    nc = bacc.Bacc("TRN2", target_bir_lowering=False, debug=False, num_devices=8)

    salt_in = nc.declare_dram_parameter(
        "salt", [1, _src_salt() + 509 * (repeats - 1) + 13 * capt], f32, isOutput=False
    )
    x_in = nc.declare_dram_parameter("x", [NT + 1, C], f32, isOutput=False)
    xs_in = nc.declare_dram_parameter("xs", [8 * P, C], f32, isOutput=False)
    gw_in = nc.declare_dram_parameter("gw", [P, CB * E], f32, isOutput=False)
    w1_in = nc.declare_dram_parameter("w1t", [P, FB * CB * P], bf16, isOutput=False)
    w2_in = nc.declare_dram_parameter("w2t", [FB, P, C], bf16, isOutput=False)
    ident_in = nc.declare_dram_parameter("ident", [P, P], f32, isOutput=False)
    tokid1_in = nc.declare_dram_parameter("tokid1", [P, NTILE], f32, isOutput=False)
    eown_in = nc.declare_dram_parameter("eown", [P, 1], f32, isOutput=False)
    ylo = nc.declare_dram_parameter("ylo", [NT + 1, CH], f32, isOutput=True)
    yhi = nc.declare_dram_parameter("yhi", [NT + 1, CH], f32, isOutput=True)
    youts = [ylo, yhi]

    with tile.TileContext(nc) as tc, ExitStack() as ctx:
        pers = ctx.enter_context(tc.tile_pool(name="pers", bufs=1))
        rt = ctx.enter_context(tc.tile_pool(name="rt", bufs=3))
        xap = ctx.enter_context(tc.tile_pool(name="xa", bufs=2))
        xgp = ctx.enter_context(tc.tile_pool(name="xg", bufs=2))
        xtp = ctx.enter_context(tc.tile_pool(name="xt", bufs=1))
        xgtp = ctx.enter_context(tc.tile_pool(name="xgt", bufs=1))
        htp = ctx.enter_context(tc.tile_pool(name="ht", bufs=1))
        yap = ctx.enter_context(tc.tile_pool(name="yap", bufs=2))
        psp = ctx.enter_context(tc.tile_pool(name="ps", bufs=2, space="PSUM"))
        # transposes get their own 3-deep PSUM ring so the per-tile router
        # cadence isn't limited by the copy-out of two tiles ago
        tpp = ctx.enter_context(tc.tile_pool(name="tp", bufs=3, space="PSUM"))
        dram = ctx.enter_context(tc.tile_pool(name="dram", bufs=1, space="DRAM"))

        from concourse.tile_rust import add_dep_helper

        for _rep in range(repeats):
            lib_inst = nc.gpsimd.load_library(library_config.sparse_gather)

            ident = pers.tile([P, P], f32)
            nc.sync.dma_start(ident[:], ident_in[:])
            gw_sb = pers.tile([P, CB * E], f32)
            nc.sync.dma_start(gw_sb[:], gw_in[:])
            tokid1 = pers.tile([P, NTILE], f32)
            eown = pers.tile([P, 1], f32)
            # expert weights: resident in SBUF for the whole kernel. The loads
            # are issued AFTER the router's x reads (below) so they don't hog
            # the DMA engines ahead of the router's critical path.
            w1res = pers.tile([P, FB * CB * P], bf16)
            w2res = pers.tile([P, FB, C], bf16)

            # ---------------- router (f32 exact), distributed ----------------
            # each core routes its own 1024-token slice (xs), then one
            # AllGather exchanges packed (top1, top2, idx1, idx2) per token
            M8 = pers.tile([P, 8, 8], f32)
            I8 = pers.tile([P, 8, 8], u32)
            last_xa = None
            for j in range(8):
                xa = xap.tile([P, C], f32, tag="xa")
                last_xa = nc.sync.dma_start(xa[:], xs_in[j * P : (j + 1) * P, :])
                xT = xtp.tile([P, CB * P], f32, tag="xT")
                # transpose 4 c-blocks into one PSUM bank, copy out in two
                # [128,512] chunks (DVE + scalar engine, one each)
                for half in range(2):
                    tp = tpp.tile([P, 4 * P], f32, space="PSUM", tag="T")
                    for b4 in range(4):
                        b = half * 4 + b4
                        nc.tensor.transpose(
                            tp[:, b4 * P : (b4 + 1) * P],
                            xa[:, b * P : (b + 1) * P],
                            ident[:],
                        )
                    nc.vector.tensor_copy(xT[:, half * 4 * P : (half + 1) * 4 * P], tp[:])
                L = psp.tile([P, 8], f32, space="PSUM", tag="Y")
                for b in range(CB):
                    nc.tensor.matmul(
                        L[:],
                        lhsT=xT[:, b * P : (b + 1) * P],
                        rhs=gw_sb[:, b * E : b * E + E],
                        start=(b == 0),
                        stop=(b == CB - 1),
                    )
                Lc = rt.tile([P, 8], f32, tag="Lc")
                nc.vector.tensor_copy(Lc[:], L[:])
                nc.vector.max_with_indices(M8[:, j, :], I8[:, j, :], Lc[:])

            # small router-math inputs: loaded after the slice reads
            nc.sync.dma_start(tokid1[:], tokid1_in[:])
            nc.sync.dma_start(eown[:], eown_in[:])

            # pack (top1, top2, idx1, idx2) and AllGather across the 8 cores
            P4 = pers.tile([P, 8, 4], f32)
            nc.vector.tensor_copy(P4[:, :, 0:1], M8[:, :, 0:1])
            nc.vector.tensor_copy(P4[:, :, 1:2], M8[:, :, 1:2])
            nc.vector.tensor_copy(P4[:, :, 2:3], I8[:, :, 0:1])
            nc.vector.tensor_copy(P4[:, :, 3:4], I8[:, :, 1:2])
            cc_in = dram.tile([P, 32], f32, tag="ccin")
            ccd = nc.sync.dma_start(cc_in[:], P4[:].rearrange("p a b -> p (a b)"))
            # w1 load: sequenced behind the collective's input DMA and chunked
            # so the G4 readback and compaction bounces can slot between
            # chunks on the exclusive DMA engines. w2 is issued later (in the
            # MLP section) interleaved with the first gathers.
            W1CH = 16
            w1_tail = []
            for wq in range(W1CH):
                cw = FB * CB * P // W1CH
                wd1 = nc.scalar.dma_start(
                    w1res[:, wq * cw : (wq + 1) * cw],
                    w1_in[:, wq * cw : (wq + 1) * cw],
                )
                if wq < 12:
                    # these fill the DMA engines during the collective window
                    add_dep_helper(wd1.ins, ccd.ins, reason="collective input first")
                else:
                    # deferred until the G4 readback so it isn't FIFO-starved
                    w1_tail.append(wd1)
            cc_out = dram.tile([8 * P, 32], f32, tag="ccout")
            cc_inst = nc.gpsimd.collective_compute(
                "AllGather",
                mybir.AluOpType.bypass,
                replica_groups=[list(range(E))],
                ins=[cc_in[:].opt()],
                outs=[cc_out[:].opt()],
            )
            add_dep_helper(cc_inst.ins, ccd.ins, reason="collective after input write")
            # G4[p, rank, t, f] = cc_out[rank*128+p, t*4+f]; global tile
            # j = rank*8 + t matches token id = j*128 + p
            G4 = pers.tile([P, 8, 32], f32)
            g4_dma = nc.sync.dma_start(
                G4[:], cc_out[:].rearrange("(r p) q -> p r q", r=8)
            )
            add_dep_helper(g4_dma.ins, cc_inst.ins, reason="readback after collective")
            for wd1 in w1_tail:
                add_dep_helper(wd1.ins, g4_dma.ins, reason="G4 readback first")
            # field f of global tile j = rank*8+t lives at G4[p, rank, t*4+f]
            g4f = lambda f: G4[:, :, f::4].rearrange("p a b -> p (a b)")

            # batched router math -> per-token selection + combine weight for e_own
            CW1 = pers.tile([P, NTILE], f32)
            SEL = pers.tile([P, NTILE], f32)
            val1 = pers.tile([P, NTILE], f32)
            val2 = pers.tile([P, NTILE], f32)
            d = rt.tile([P, NTILE], f32, tag="rm")
            nc.vector.tensor_tensor(d[:], g4f(0), g4f(1), op=mybir.AluOpType.subtract)
            nc.scalar.activation(CW1[:], d[:], mybir.ActivationFunctionType.Sigmoid)
            se1 = rt.tile([P, NTILE], f32, tag="rm")
            se2 = rt.tile([P, NTILE], f32, tag="rm")
            # se1 = (idx1 == e_own), se2 = (idx2 == e_own)   (per-partition scalar AP)
            nc.vector.tensor_scalar(se1[:], g4f(2), eown[:, 0:1], None, op0=mybir.AluOpType.is_equal)
            nc.vector.tensor_scalar(se2[:], g4f(3), eown[:, 0:1], None, op0=mybir.AluOpType.is_equal)
            nc.vector.tensor_add(SEL[:], se1[:], se2[:])
            # CWE = se1*cw1 + se2*(1-cw1)
            t1 = rt.tile([P, NTILE], f32, tag="rm")
            t2 = rt.tile([P, NTILE], f32, tag="rm")
            CWE = rt.tile([P, NTILE], f32, tag="rm")
            nc.vector.tensor_mul(t1[:], se1[:], CW1[:])
            nc.vector.tensor_mul(t2[:], se2[:], CW1[:])
            nc.vector.tensor_sub(t2[:], se2[:], t2[:])
            nc.vector.tensor_add(CWE[:], t1[:], t2[:])
            # val1 = SEL * (tokid+1) - 1 ; val2 = SEL * (CWE+1) - 1
            nc.vector.tensor_mul(val1[:], SEL[:], tokid1[:])
            nc.vector.tensor_scalar(val1[:], val1[:], 1.0, None, op0=mybir.AluOpType.subtract)
            nc.vector.tensor_scalar(t1[:], CWE[:], 1.0, None, op0=mybir.AluOpType.add)
            nc.vector.tensor_mul(val2[:], SEL[:], t1[:])
            nc.vector.tensor_scalar(val2[:], val2[:], 1.0, None, op0=mybir.AluOpType.subtract)

            # ---------------- compaction ----------------
            v1w = pers.tile([16, NT // 16], f32)
            v2w = pers.tile([16, NT // 16], f32)
            for q in range(8):
                nc.sync.dma_start(v1w[0:16, q * NTILE : (q + 1) * NTILE], val1[16 * q : 16 * (q + 1), :])
                nc.sync.dma_start(v2w[0:16, q * NTILE : (q + 1) * NTILE], val2[16 * q : 16 * (q + 1), :])
            idsC = pers.tile([16, capw], f32)
            cwC = pers.tile([16, capw], f32)
            nf1 = pers.tile([1, 1], u32)
            nf2 = pers.tile([1, 1], u32)
            # HW sparse_gather writes only the found prefix; pre-fill the tail
            # marker ourselves (the simulator fills -1, hardware does not).
            nc.vector.memset(idsC[:], -1.0)
            nc.vector.memset(cwC[:], -1.0)
            sg1 = nc.gpsimd.sparse_gather(idsC[:], v1w[:], num_found=nf1[:])
            sg2 = nc.gpsimd.sparse_gather(cwC[:], v2w[:], num_found=nf2[:])
            add_dep_helper(sg1.ins, lib_inst.ins, reason="sparse_gather needs library")
            add_dep_helper(sg2.ins, lib_inst.ins, reason="sparse_gather needs library")
            # tail fill is -1: ids -> DUMP, cw -> 0
            neg = rt.tile([16, capw], f32, tag="fix")
            nc.vector.tensor_scalar(neg[:], idsC[:], 0.0, None, op0=mybir.AluOpType.is_lt)
            nc.vector.scalar_tensor_tensor(
                idsC[:], neg[:], float(DUMP + 1), idsC[:],
                op0=mybir.AluOpType.mult, op1=mybir.AluOpType.add,
            )
            nc.vector.tensor_scalar_max(cwC[:], cwC[:], 0.0)

            IDS = pers.tile([P, capt], f32)
            CWS = pers.tile([P, capt], f32)
            for r in range(8):
                nc.sync.dma_start(IDS[16 * r : 16 * (r + 1), :], idsC[0:16, r::8])
                cws_dma = nc.sync.dma_start(CWS[16 * r : 16 * (r + 1), :], cwC[0:16, r::8])
            IDSi = pers.tile([P, capt], i32)
            nc.vector.tensor_copy(IDSi[:], IDS[:])

            # ---------------- expert MLP over capacity passes ----------------
            k0 = 0  # running gather-tile index
            w2_loaded = False
            for p, wt in enumerate(widths):
                w = wt * P
                xgT = xgtp.tile([P, CB, w], bf16, tag="xgT", name=f"xgT_{_rep}_{p}")
                for g in range(wt):
                    k = k0 + g
                    xg = xgp.tile([P, C], f32, tag="xg")
                    gth = nc.gpsimd.indirect_dma_start(
                        out=xg[:],
                        out_offset=None,
                        in_=x_in[:],
                        in_offset=bass.IndirectOffsetOnAxis(ap=IDSi[:, k : k + 1], axis=0),
                    )
                    last_gth = gth  # SWDGE prep only; transfer ordering via transposes
                    for half in range(2):
                        tp = tpp.tile([P, 4 * P], f32, space="PSUM", tag="T")
                        for b4 in range(4):
                            b = half * 4 + b4
                            nc.tensor.transpose(
                                tp[:, b4 * P : (b4 + 1) * P],
                                xg[:, b * P : (b + 1) * P],
                                ident[:],
                            )
                        # strided copy into xgT (downcast to bf16)
                        dst = xgT[:, half * 4 : half * 4 + 4, g * P : (g + 1) * P]
                        srcv = tp[:].rearrange("p (b t) -> p b t", b=4)
                        last_xgt_copy = nc.vector.tensor_copy(dst, srcv)

                if not w2_loaded:
                    # w2 load: one DMA, sequenced behind pass-0's gathers so
                    # it cannot starve them on the exclusive DMA engines; it
                    # completes well before the first down-proj needs it
                    wd2 = nc.scalar.dma_start(
                        w2res[:], w2_in[:].rearrange("f p c -> p f c")
                    )
                    # the SWDGE gather splits prep from transfer; depending on
                    # the xgT copy (which reads the gathered data) is the only
                    # way to order w2's transfer behind the gather transfers
                    add_dep_helper(wd2.ins, last_xgt_copy.ins, reason="gathers first")
                    w2_loaded = True

                hT = htp.tile([P, FB, w], bf16, tag="hT", name=f"hT_{_rep}_{p}")
                for fb in range(FB):
                    psA = psp.tile([P, w], f32, space="PSUM", tag="A")
                    for b in range(CB):
                        nc.tensor.matmul(
                            psA[:],
                            lhsT=w1res[:, (fb * CB + b) * P : (fb * CB + b + 1) * P],
                            rhs=xgT[:, b, :],
                            start=(b == 0),
                            stop=(b == CB - 1),
                        )
                    nc.scalar.activation(hT[:, fb, :], psA[:], mybir.ActivationFunctionType.Gelu)

                # token-major down-proj: psY[tok, c-half] = sum_fb hT_blk.T @ w2_blk
                for g in range(wt):
                    k = k0 + g
                    for h in range(2):
                        psY = psp.tile([P, CH], f32, space="PSUM", tag="Y")
                        for fb in range(FB):
                            nc.tensor.matmul(
                                psY[:],
                                lhsT=hT[:, fb, g * P : (g + 1) * P],
                                rhs=w2res[:, fb, h * CH : (h + 1) * CH],
                                start=(fb == 0),
                                stop=(fb == FB - 1),
                            )
                        yasm = yap.tile([P, CH], f32, tag="yasm")
                        nc.vector.tensor_scalar_mul(yasm[:], psY[:], CWS[:, k : k + 1])
                        # scatter-ADD 128 token rows into the half-width table.
                        # out is a 128-row prefix of the table: the indirect row
                        # offset addresses the full [NT+1, CH] tensor (coef =
                        # CH from the AP shape), and 128 rows is the true
                        # transfer size. Dump slots carry exact +/-0 rows and
                        # row stride is 2KB so the max DGE byte offset is
                        # 16.8MB < 2^25: no offset wrap anywhere. Real token
                        # rows are unique per core and the output buffer starts
                        # zeroed, so add == write for them.
                        nc.gpsimd.indirect_dma_start(
                            out=youts[h][0:P, :],
                            out_offset=bass.IndirectOffsetOnAxis(ap=IDSi[:, k : k + 1], axis=0),
                            in_=yasm[:],
                            in_offset=None,
                            compute_op=mybir.AluOpType.add,
                        )
                k0 += wt

    nc.compile()
    return nc


def prep_inputs(x, gate_w, w1, w2):
    """Host-side input prep (layout/dtype only). Returns per-core input maps."""
    import ml_dtypes

    global _CAP
    if _CAP is None:
        _CAP = compute_cap(np.asarray(x, np.float32), gate_w)
    x = np.ascontiguousarray(np.asarray(x, dtype=np.float32)).reshape(NT, C)
    gate_w = np.asarray(gate_w, dtype=np.float32)
    w1 = np.asarray(w1, dtype=np.float32)
    w2 = np.asarray(w2, dtype=np.float32)

    x_pad = np.zeros((NT + 1, C), dtype=np.float32)
    x_pad[:NT] = x

    gw = np.ascontiguousarray(
        gate_w.reshape(CB, P, E).transpose(1, 0, 2).reshape(P, CB * E)
    )
    ident = np.eye(P, dtype=np.float32)
    tokid1 = (np.arange(NT, dtype=np.float32).reshape(NTILE, P).T + 1.0).copy()

    in_maps = []
    for e in range(E):
        # w1t: [C,F] -> [cb, pc, fb, pf] -> [pc, fb, cb, pf] -> [P, FB*CB*P]
        w1t = np.ascontiguousarray(
            w1[e].reshape(CB, P, FB, P).transpose(1, 2, 0, 3).reshape(P, FB * CB * P)
        ).astype(ml_dtypes.bfloat16)
        # w2t: [F,C] -> [FB, P, C]
        w2t = np.ascontiguousarray(w2[e].reshape(FB, P, C)).astype(ml_dtypes.bfloat16)
        in_maps.append(
            {
                "salt": np.zeros((1, _src_salt() + 13 * (_CAP // P)), dtype=np.float32),
                "x": x_pad,
                "xs": np.ascontiguousarray(x_pad[e * 8 * P : (e + 1) * 8 * P]),
                "gw": gw,
                "w1t": w1t,
                "w2t": w2t,
                "ident": ident,
                "tokid1": tokid1,
                "eown": np.full((P, 1), float(e), dtype=np.float32),
            }
        )
    return in_maps


def compute_cap(x, gate_w):
    """Capacity from the actual routing: max over experts of the token count,
    and (in case sparse_gather compacts per 16-partition row rather than
    globally) 16x the max per-(expert, token%16) count. Rounded up to a
    multiple of 128."""
    logits = x.reshape(NT, C).astype(np.float32) @ np.asarray(gate_w, np.float32)
    top2 = np.argpartition(-logits, 2, axis=1)[:, :2]
    counts = np.bincount(top2.ravel(), minlength=E)
    row = np.arange(NT) % 16
    need = counts.max()
    for e in range(E):
        sel = (top2 == e).any(axis=1)
        rc = np.bincount(row[sel], minlength=16)
        need = max(need, 16 * rc.max())
    return int(-(-int(need) // P) * P)


_NC = None
_CAP = None


def _run_with_retries(nc, in_maps, attempts=4):
    """The first execution of a freshly-compiled NEFF occasionally fails with a
    transient runtime error (executable-load race in the remote terminal);
    subsequent executions succeed. Retry with backoff."""
    import time as _time

    last = None
    for i in range(attempts):
        try:
            return run_bass_kernel_spmd(nc, in_maps, list(range(E)))
        except Exception as e:  # jax.errors.JaxRuntimeError and friends
            last = e
            _time.sleep(5 + 15 * i)
    raise last



# ---- constants for the safe fallback path (v1 baseline program) ----
_S_CAP = 2560
_S_PASS_N = 512
_S_NPASS = _S_CAP // _S_PASS_N
_S_CAPT = _S_CAP // P
_S_CAPW = _S_CAP // 16
f32r = mybir.dt.float32r

def _build_nc_safe(repeats=1, mm=None):
    """repeats: trace the whole program body N times back-to-back (straight-
    line); used to measure per-invocation HW time by differencing."""
    mm = mm or "f32"
    mdt = f32 if mm == "f32" else bf16
    nc = bacc.Bacc("TRN2", target_bir_lowering=False, debug=False, num_devices=8)

    salt_in = nc.declare_dram_parameter(
        "salt",
        [1, _src_salt() + 509 * (repeats - 1) + 2039 * (mm != "f32")],
        f32,
        isOutput=False,
    )
    x_in = nc.declare_dram_parameter("x", [NT + 1, C], f32, isOutput=False)
    gw_in = nc.declare_dram_parameter("gw", [P, CB * E], f32, isOutput=False)
    w1_in = nc.declare_dram_parameter("w1t", [FB, P, CB, P], mdt, isOutput=False)
    w2_in = nc.declare_dram_parameter("w2t", [CB, P, FB, P], mdt, isOutput=False)
    ident_in = nc.declare_dram_parameter("ident", [P, P], f32, isOutput=False)
    tokid1_in = nc.declare_dram_parameter("tokid1", [P, NTILE], f32, isOutput=False)
    eown_in = nc.declare_dram_parameter("eown", [P, 1], f32, isOutput=False)
    yout = nc.declare_dram_parameter("yout", [NT + 1, C], f32, isOutput=True)

    with tile.TileContext(nc) as tc, ExitStack() as ctx:
        pers = ctx.enter_context(tc.tile_pool(name="pers", bufs=1))
        rt = ctx.enter_context(tc.tile_pool(name="rt", bufs=3))
        xap = ctx.enter_context(tc.tile_pool(name="xa", bufs=3))
        xtp = ctx.enter_context(tc.tile_pool(name="xt", bufs=2))
        xgtp = ctx.enter_context(tc.tile_pool(name="xgt", bufs=1))
        htp = ctx.enter_context(tc.tile_pool(name="ht", bufs=1))
        w1p = ctx.enter_context(tc.tile_pool(name="w1p", bufs=3))
        w2p = ctx.enter_context(tc.tile_pool(name="w2p", bufs=2))
        ytp = ctx.enter_context(tc.tile_pool(name="ytp", bufs=2))
        yap = ctx.enter_context(tc.tile_pool(name="yap", bufs=5))
        psp = ctx.enter_context(tc.tile_pool(name="ps", bufs=2, space="PSUM"))

        for _rep in range(repeats):
            lib_inst = nc.gpsimd.load_library(library_config.sparse_gather)

            ident = pers.tile([P, P], f32)
            nc.sync.dma_start(ident[:], ident_in[:])
            gw_sb = pers.tile([P, CB * E], f32)
            nc.sync.dma_start(gw_sb[:], gw_in[:])
            tokid1 = pers.tile([P, NTILE], f32)
            nc.sync.dma_start(tokid1[:], tokid1_in[:])
            eown = pers.tile([P, 1], f32)
            nc.sync.dma_start(eown[:], eown_in[:])

            # ---------------- router ----------------
            M8 = pers.tile([P, NTILE, 8], f32)
            I8 = pers.tile([P, NTILE, 8], u32)
            for j in range(NTILE):
                xa = xap.tile([P, C], f32, tag="xa")
                nc.sync.dma_start(xa[:], x_in[j * P : (j + 1) * P, :])
                xT = xtp.tile([P, CB, P], f32, tag="xT")
                for b in range(CB):
                    tp = psp.tile([P, P], f32, space="PSUM", tag="T")
                    nc.tensor.transpose(tp[:], xa[:, b * P : (b + 1) * P], ident[:])
                    nc.vector.tensor_copy(xT[:, b, :], tp[:])
                L = psp.tile([P, 8], f32, space="PSUM", tag="Y")
                for b in range(CB):
                    nc.tensor.matmul(
                        L[:],
                        lhsT=xT[:, b, :],
                        rhs=gw_sb[:, b * E : b * E + E],
                        start=(b == 0),
                        stop=(b == CB - 1),
                    )
                Lc = rt.tile([P, 8], f32, tag="Lc")
                nc.vector.tensor_copy(Lc[:], L[:])
                nc.vector.max_with_indices(M8[:, j, :], I8[:, j, :], Lc[:])

            # batched router math -> per-token selection + combine weight for e_own
            CW1 = pers.tile([P, NTILE], f32)
            SEL = pers.tile([P, NTILE], f32)
            val1 = pers.tile([P, NTILE], f32)
            val2 = pers.tile([P, NTILE], f32)
            d = rt.tile([P, NTILE], f32, tag="rm")
            nc.vector.tensor_tensor(d[:], M8[:, :, 0], M8[:, :, 1], op=mybir.AluOpType.subtract)
            nc.scalar.activation(CW1[:], d[:], mybir.ActivationFunctionType.Sigmoid)
            if1 = rt.tile([P, NTILE], f32, tag="rm")
            if2 = rt.tile([P, NTILE], f32, tag="rm")
            nc.vector.tensor_copy(if1[:], I8[:, :, 0])
            nc.vector.tensor_copy(if2[:], I8[:, :, 1])
            se1 = rt.tile([P, NTILE], f32, tag="rm")
            se2 = rt.tile([P, NTILE], f32, tag="rm")
            # se1 = (if1 == e_own), se2 = (if2 == e_own)   (per-partition scalar AP)
            nc.vector.tensor_scalar(se1[:], if1[:], eown[:, 0:1], None, op0=mybir.AluOpType.is_equal)
            nc.vector.tensor_scalar(se2[:], if2[:], eown[:, 0:1], None, op0=mybir.AluOpType.is_equal)
            nc.vector.tensor_add(SEL[:], se1[:], se2[:])
            # CWE = se1*cw1 + se2*(1-cw1)
            t1 = rt.tile([P, NTILE], f32, tag="rm")
            t2 = rt.tile([P, NTILE], f32, tag="rm")
            CWE = rt.tile([P, NTILE], f32, tag="rm")
            nc.vector.tensor_mul(t1[:], se1[:], CW1[:])
            nc.vector.tensor_mul(t2[:], se2[:], CW1[:])
            nc.vector.tensor_sub(t2[:], se2[:], t2[:])
            nc.vector.tensor_add(CWE[:], t1[:], t2[:])
            # val1 = SEL * (tokid+1) - 1 ; val2 = SEL * (CWE+1) - 1
            nc.vector.tensor_mul(val1[:], SEL[:], tokid1[:])
            nc.vector.tensor_scalar(val1[:], val1[:], 1.0, None, op0=mybir.AluOpType.subtract)
            nc.vector.tensor_scalar(t1[:], CWE[:], 1.0, None, op0=mybir.AluOpType.add)
            nc.vector.tensor_mul(val2[:], SEL[:], t1[:])
            nc.vector.tensor_scalar(val2[:], val2[:], 1.0, None, op0=mybir.AluOpType.subtract)

            # ---------------- compaction ----------------
            v1w = pers.tile([16, NT // 16], f32)
            v2w = pers.tile([16, NT // 16], f32)
            for q in range(8):
                nc.sync.dma_start(v1w[0:16, q * NTILE : (q + 1) * NTILE], val1[16 * q : 16 * (q + 1), :])
                nc.sync.dma_start(v2w[0:16, q * NTILE : (q + 1) * NTILE], val2[16 * q : 16 * (q + 1), :])
            idsC = pers.tile([16, _S_CAPW], f32)
            cwC = pers.tile([16, _S_CAPW], f32)
            nf1 = pers.tile([1, 1], u32)
            nf2 = pers.tile([1, 1], u32)
            # HW sparse_gather writes only the found prefix; pre-fill the tail
            # marker ourselves (the simulator fills -1, hardware does not).
            nc.vector.memset(idsC[:], -1.0)
            nc.vector.memset(cwC[:], -1.0)
            sg1 = nc.gpsimd.sparse_gather(idsC[:], v1w[:], num_found=nf1[:])
            sg2 = nc.gpsimd.sparse_gather(cwC[:], v2w[:], num_found=nf2[:])
            from concourse.tile_rust import add_dep_helper

            add_dep_helper(sg1.ins, lib_inst.ins, reason="sparse_gather needs library")
            add_dep_helper(sg2.ins, lib_inst.ins, reason="sparse_gather needs library")
            # tail fill is -1: ids -> DUMP, cw -> 0
            neg = rt.tile([16, _S_CAPW], f32, tag="fix")
            nc.vector.tensor_scalar(neg[:], idsC[:], 0.0, None, op0=mybir.AluOpType.is_lt)
            nc.vector.scalar_tensor_tensor(
                idsC[:], neg[:], float(DUMP + 1), idsC[:],
                op0=mybir.AluOpType.mult, op1=mybir.AluOpType.add,
            )
            nc.vector.tensor_scalar_max(cwC[:], cwC[:], 0.0)

            IDS = pers.tile([P, _S_CAPT], f32)
            CWS = pers.tile([P, _S_CAPT], f32)
            for r in range(8):
                nc.sync.dma_start(IDS[16 * r : 16 * (r + 1), :], idsC[0:16, r::8])
                nc.sync.dma_start(CWS[16 * r : 16 * (r + 1), :], cwC[0:16, r::8])
            IDSi = pers.tile([P, _S_CAPT], i32)
            nc.vector.tensor_copy(IDSi[:], IDS[:])

            # ---------------- expert MLP over capacity passes ----------------
            for p in range(_S_NPASS):
                xgT = xgtp.tile([P, CB, _S_PASS_N], mdt, tag="xgT")
                for g in range(_S_PASS_N // P):
                    k = p * (_S_PASS_N // P) + g
                    xg = xap.tile([P, C], f32, tag="xa")
                    nc.gpsimd.indirect_dma_start(
                        out=xg[:],
                        out_offset=None,
                        in_=x_in[:],
                        in_offset=bass.IndirectOffsetOnAxis(ap=IDSi[:, k : k + 1], axis=0),
                    )
                    for b in range(CB):
                        tp = psp.tile([P, P], f32, space="PSUM", tag="T")
                        nc.tensor.transpose(tp[:], xg[:, b * P : (b + 1) * P], ident[:])
                        nc.vector.tensor_copy(xgT[:, b, g * P : (g + 1) * P], tp[:])

                hT = htp.tile([P, FB, _S_PASS_N], mdt, tag="hT")
                for fb in range(FB):
                    w1sb = w1p.tile([P, CB * P], mdt, tag="w1")
                    nc.sync.dma_start(w1sb[:], w1_in[fb].rearrange("c b f -> c (b f)"))
                    psA = psp.tile([P, _S_PASS_N], f32, space="PSUM", tag="A")
                    for b in range(CB):
                        nc.tensor.matmul(
                            psA[:],
                            lhsT=w1sb[:, b * P : (b + 1) * P],
                            rhs=xgT[:, b, :],
                            start=(b == 0),
                            stop=(b == CB - 1),
                        )
                    nc.scalar.activation(hT[:, fb, :], psA[:], mybir.ActivationFunctionType.Gelu)

                yasm = [
                    yap.tile([P, C], f32, tag="yasm", name=f"yasm_{_rep}_{p}_{g}")
                    for g in range(_S_PASS_N // P)
                ]
                for cb in range(CB):
                    w2sb = w2p.tile([P, FB * P], mdt, tag="w2")
                    nc.sync.dma_start(w2sb[:], w2_in[cb].rearrange("f b c -> f (b c)"))
                    psY = psp.tile([P, _S_PASS_N], f32, space="PSUM", tag="Y")
                    for fb in range(FB):
                        nc.tensor.matmul(
                            psY[:],
                            lhsT=w2sb[:, fb * P : (fb + 1) * P],
                            rhs=hT[:, fb, :],
                            start=(fb == 0),
                            stop=(fb == FB - 1),
                        )
                    yT = ytp.tile([P, _S_PASS_N], f32, tag="yT")
                    nc.vector.tensor_copy(yT[:], psY[:])
                    for g in range(_S_PASS_N // P):
                        k = p * (_S_PASS_N // P) + g
                        tp = psp.tile([P, P], f32, space="PSUM", tag="T")
                        nc.tensor.transpose(tp[:], yT[:, g * P : (g + 1) * P], ident[:])
                        nc.vector.tensor_scalar_mul(
                            yasm[g][:, cb * P : (cb + 1) * P], tp[:], CWS[:, k : k + 1]
                        )
                for g in range(_S_PASS_N // P):
                    k = p * (_S_PASS_N // P) + g
                    # scatter-ADD: dump slots carry exact +/-0 rows, so wherever
                    # the hardware lands the out-of-range dump index (row 0 via the
                    # 25-bit DGE offset wrap; row NT in the simulator), adding
                    # zeros is harmless. Real token rows are unique per core and
                    # the output buffer starts zeroed, so add == write for them.
                    nc.gpsimd.indirect_dma_start(
                        out=yout[:],
                        out_offset=bass.IndirectOffsetOnAxis(ap=IDSi[:, k : k + 1], axis=0),
                        in_=yasm[g][:],
                        in_offset=None,
                        compute_op=mybir.AluOpType.add,
                    )

    nc.compile()
    return nc



def _prep_inputs_safe(x, gate_w, w1, w2, mm=None):
    """Host-side input prep. Returns per-core input maps."""
    import ml_dtypes

    mm = mm or "f32"
    wdt = np.float32 if mm == "f32" else ml_dtypes.bfloat16
    x = np.ascontiguousarray(np.asarray(x, dtype=np.float32)).reshape(NT, C)
    gate_w = np.asarray(gate_w, dtype=np.float32)
    w1 = np.asarray(w1, dtype=np.float32)
    w2 = np.asarray(w2, dtype=np.float32)

    x_pad = np.zeros((NT + 1, C), dtype=np.float32)
    x_pad[:NT] = x
    gw = np.ascontiguousarray(
        gate_w.reshape(CB, P, E).transpose(1, 0, 2).reshape(P, CB * E)
    )
    ident = np.eye(P, dtype=np.float32)
    tokid1 = (np.arange(NT, dtype=np.float32).reshape(NTILE, P).T + 1.0).copy()

    in_maps = []
    for e in range(E):
        w1t = np.ascontiguousarray(
            w1[e].reshape(CB, P, FB, P).transpose(2, 1, 0, 3).astype(wdt)
        )
        w2t = np.ascontiguousarray(
            w2[e].reshape(FB, P, CB, P).transpose(2, 1, 0, 3).astype(wdt)
        )
        in_maps.append(
            {
                "salt": np.zeros((1, _src_salt()), dtype=np.float32),
                "x": x_pad,
                "gw": gw,
                "w1t": w1t,
                "w2t": w2t,
                "ident": ident,
                "tokid1": tokid1,
                "eown": np.full((P, 1), float(e), dtype=np.float32),
            }
        )
    return in_maps



def _output_suspect(out, x, gate_w, w1, w2):
    if (not np.all(np.isfinite(out))) or np.abs(out).max() > 1e3:
        return True
    # spot-check a few tokens against the exact host-computed MoE output
    xf = np.asarray(x, np.float32).reshape(NT, C)
    gwf = np.asarray(gate_w, np.float32)
    from scipy.special import erf  # exact gelu; scipy ships with the env

    def gelu(v):
        return 0.5 * v * (1.0 + erf(v / np.sqrt(2.0)))

    rng_rows = [1, NT // 3, NT // 2 + 7, NT - 2]
    scale = max(np.abs(out).max(), 1e-6)
    for t in rng_rows:
        lg = xf[t] @ gwf
        i2 = np.argsort(-lg)[:2]
        wts = np.exp(lg[i2] - lg[i2].max())
        wts = wts / wts.sum()
        y = np.zeros(C, np.float32)
        for wgt, e in zip(wts, i2):
            h = gelu(xf[t] @ np.asarray(w1[e], np.float32))
            y += np.float32(wgt) * (h @ np.asarray(w2[e], np.float32))
        if np.abs(out[t] - y).max() / scale > 5e-2:
            return True
    return False


def kernel(x, gate_w, w1, w2):
    global _NC, _CAP
    x = np.asarray(x)
    if _NC is None:
        _CAP = compute_cap(np.asarray(x, np.float32), gate_w)
        _NC = build_nc(_CAP)
    in_maps = prep_inputs(x, gate_w, w1, w2)
    try:
        res = _run_with_retries(_NC, in_maps)
        out = np.zeros((NT, C), dtype=np.float32)
        for e in range(E):
            out[:, :CH] += res.results[e]["ylo"][:NT]
            out[:, CH:] += res.results[e]["yhi"][:NT]
        suspect = _output_suspect(out, x, gate_w, w1, w2)
    except Exception:
        suspect = True
    if suspect:
        # fall back to the proven baseline program (correctness first)
        import time as _time
        _time.sleep(30)
        _NC = _build_nc_safe()
        in_maps = _prep_inputs_safe(x, gate_w, w1, w2)
        res = _run_with_retries(_NC, in_maps)
        out = np.zeros((NT, C), dtype=np.float32)
        for e in range(E):
            out += res.results[e]["yout"][:NT]
    return out.reshape(B, T, C)
